# revision 1
# baseline (speedup 1.0000x reference)
"""DiscreteMMSE Trainium2 kernel.

Math (per batch b):
  W = task_pool[:,:,0]                      # (T, D)
  pred[t,p] = W[t] . x[p]                  # layout B: tasks on partitions
  err  = pred - y          (via K=65 augmented matmul: [W | -1] x [X ; y])
  sq   = err^2             (ACT Square)
  C    = exclusive cumsum_p sq   (DVE tensor_tensor_scan, mask-reset segments)
  m[p] = min_t C[t,p]      (DVE chunk min chain + PE transposes)
  Cs   = C - m             (DVE subtract; 8 of 64 chunks via PE ident+rank-1
                            matmuls into PSUM to balance engine load)
  e    = exp(-0.5 Cs)      (ACT)
  num/den: e-chunk slices (stationary) x [W | 1] (moving, N=65) accumulate
           ws[j] = (128 p-lanes, 65) in PSUM over the 32 task chunks (PE)
  out[p] = (x[p] . ws[.., 0:64]) / ws[.., 64]   (lane-parallel epilogue)
The per-column shift by m cancels exactly in num/den ratio; softmax over
4096 tasks is reproduced bit-closely (fp32 throughout).

Sharding: data-parallel over batch: 32 batches -> 8 cores x 4. No collectives.
"""

import os
import sys

sys.path.insert(0, "/opt/trn_rl_repo")
sys.path.insert(0, "/opt/trn_rl_repo/concourse")

import numpy as np

import concourse.bass as bass
import concourse.tile as tile
from concourse import bacc, bass_utils, mybir

F32 = mybir.dt.float32
F32R = mybir.dt.float32r
AF = mybir.ActivationFunctionType
ALU = mybir.AluOpType
AX = mybir.AxisListType

B, P, D, T = 32, 256, 64, 4096
NCORES = 8
BLOC = B // NCORES          # 4 batches per core
NSEG = 2                    # batches per group (scan segments)
NG = BLOC // NSEG           # 2 groups
NCH = T // 128              # 32 task chunks
SEG = 258                   # scan segment stride: [pad, pad, sq0..sq255]
DA = D + 1                  # augmented contraction


def build_program(tc):
    nc = tc.nc

    w_dram = nc.dram_tensor("w_aug", (DA, T), F32, kind="ExternalInput").ap()
    tp_dram = nc.dram_tensor("tp_aug", (T, DA), F32, kind="ExternalInput").ap()
    xa_dram = nc.dram_tensor("x_aug", (DA, BLOC * P), F32, kind="ExternalInput").ap()
    id_dram = nc.dram_tensor("ident", (128, 128), F32, kind="ExternalInput").ap()
    xn_dram = nc.dram_tensor("x_nat", (BLOC * P, D), F32, kind="ExternalInput").ap()
    out_dram = nc.dram_tensor("out", (BLOC, P), F32, kind="ExternalOutput").ap()


    from contextlib import ExitStack

    with ExitStack() as ctx:
        consts = ctx.enter_context(tc.tile_pool(name="consts", bufs=1))
        sqp = ctx.enter_context(tc.tile_pool(name="sqp", bufs=1))
        cp = ctx.enter_context(tc.tile_pool(name="cp", bufs=40))
        ep = ctx.enter_context(tc.tile_pool(name="ep", bufs=4))
        rmp = ctx.enter_context(tc.tile_pool(name="rmp", bufs=1))
        sm = ctx.enter_context(tc.tile_pool(name="sm", bufs=2))
        errp = ctx.enter_context(tc.tile_pool(name="errp", bufs=2, space="PSUM"))
        csp = ctx.enter_context(tc.tile_pool(name="csp", bufs=1, space="PSUM"))
        wsp = ctx.enter_context(tc.tile_pool(name="wsp", bufs=1, space="PSUM"))

        # ---- constants ----
        W_sb = consts.tile([DA, T], F32, tag="wsb", name="wsb")
        XA = consts.tile([DA, BLOC * P], F32, tag="xa", name="xa")
        ID = consts.tile([128, 128], F32, tag="ident", name="ident")
        TP_sb = consts.tile([128, NCH, DA], F32, tag="tpsb", name="tpsb")
        nc.sync.dma_start(XA[:, 0 : NSEG * P], xa_dram[:, 0 : NSEG * P])
        nc.sync.dma_start(W_sb[:, 0:256], w_dram[:, 0:256])
        nc.sync.dma_start(XA[:, NSEG * P :], xa_dram[:, NSEG * P :])
        nc.sync.dma_start(ID[:], id_dram)
        nc.sync.dma_start(W_sb[:, 256:1024], w_dram[:, 256:1024])
        nc.sync.dma_start(W_sb[:, 1024:2048], w_dram[:, 1024:2048])
        nc.sync.dma_start(W_sb[:, 2048:T], w_dram[:, 2048:T])
        nc.sync.dma_start(
            TP_sb[:], tp_dram.rearrange("(c p) d -> p c d", p=128)
        )
        XN = consts.tile([128, 2 * BLOC, D], F32, tag="xn", name="xn")
        nc.sync.dma_start(XN[:], xn_dram.rearrange("(j q) d -> q j d", q=128))

        ones1 = consts.tile([1, 128], F32, tag="ones1", name="ones1")
        nc.gpsimd.memset(ones1[:], 1.0)
        warm = consts.tile([1, 128], F32, tag="warm", name="warm")
        nc.scalar.activation(warm[:], ones1[:], AF.Square)
        mask = consts.tile([128, NSEG * SEG], F32, tag="mask", name="mask")
        nc.gpsimd.memset(mask[:], 1.0)
        for s in range(NSEG):
            nc.gpsimd.memset(mask[:, s * SEG : s * SEG + 2], 0.0)

        # persistent sq ring (pad columns stay zero forever)
        sq_ring = []
        for i in range(4):
            t = sqp.tile([128, NSEG * SEG], F32, tag=f"sqr{i}", name=f"sqr{i}")
            nc.gpsimd.memset(t[:], 0.0)
            sq_ring.append(t)

        m512 = [sm.tile([1, NSEG * P], F32, tag=f"m512{g}", name=f"m512{g}") for g in range(NG)]
        mbc = [sm.tile([128, NSEG * P], F32, tag=f"mbc{g}", name=f"mbc{g}") for g in range(NG)]
        negm512 = [sm.tile([1, NSEG * P], F32, tag=f"negm{g}", name=f"negm{g}") for g in range(NG)]

        c_tiles = {}
        ws_tiles = {}
        rm_state = {}

        def p1_chunk(g, c):
            # phase 1: err -> sq -> scan(C) -> running min, one task chunk
            err = errp.tile([128, NSEG * P], F32, tag="err", name="err")
            nc.tensor.matmul(
                err[:],
                lhsT=W_sb[:, c * 128 : (c + 1) * 128],
                rhs=XA[:, g * NSEG * P : (g + 1) * NSEG * P],
                start=True,
                stop=True,
            )
            sq = sq_ring[(g * NCH + c) % 4]
            sq_view = sq[:].rearrange("p (s x) -> p s x", x=SEG)[:, :, 2 : 2 + P]
            err_view = err[:].rearrange("p (s x) -> p s x", x=P)
            nc.scalar.activation(sq_view, err_view, AF.Square, bias=0.0, scale=0.7071067811865476)

            C = cp.tile([128, NSEG * SEG], F32, tag="c", name="c")
            c_tiles[(g, c)] = C
            nc.vector.tensor_tensor_scan(
                C[:], sq[:], mask[:], 0.0, op0=ALU.add, op1=ALU.mult
            )

            rmA, nA, firstC = rm_state[g]
            if nA == 0:
                firstC = C
            elif nA == 1:
                nc.vector.tensor_tensor(rmA[1][:], firstC[:], C[:], op=ALU.min)
            else:
                nc.vector.tensor_tensor(
                    rmA[nA % 2][:], rmA[(nA + 1) % 2][:], C[:], op=ALU.min
                )
            rm_state[g] = (rmA, nA + 1, firstC)

        def finalize(g):
            # partition-min via PE transposes, then broadcast m into SBUF
            rmA, nA, _ = rm_state[g]
            rmF = rmA[(nA + 1) % 2]
            for s in range(NSEG):
                for h in range(2):
                    blk = rmF[:, 1 + s * SEG + h * 128 : 1 + s * SEG + (h + 1) * 128]
                    tps = errp.tile([128, 128], F32, tag="err", name="err")
                    nc.tensor.transpose(tps[:], blk, ID[:])
                    mcol = sm.tile([128, 1], F32, tag="mcol", name="mcol")
                    nc.vector.tensor_reduce(mcol[:], tps[:], axis=AX.X, op=ALU.min)
                    mrow = errp.tile([1, 128], F32, tag="err", name="err")
                    nc.tensor.transpose(mrow[:], mcol[:], ID[:])
                    nc.scalar.copy(
                        m512[g][:, s * P + h * 128 : s * P + (h + 1) * 128],
                        mrow[:],
                    )
            nc.scalar.mul(negm512[g][:], m512[g][:], -1.0)
            mbp = errp.tile([128, NSEG * P], F32, tag="err", name="mbp")
            nc.tensor.matmul(mbp[:], lhsT=ones1[:], rhs=m512[g][:], start=True, stop=True)
            nc.scalar.copy(mbc[g][:], mbp[:])
            ws_tiles[g] = [
                wsp.tile([128, DA], F32, tag=f"wsj{j}", name=f"wsj{j}")
                for j in range(2 * NSEG)
            ]

        def p2_pair(g, cc):
            # phase 2: Cs = C - m (DVE), e = exp(-0.5 Cs) (ACT), ws matmul (PE)
            ws = ws_tiles[g]  # list of 4 (128, 65) psum accumulators
            mv = mbc[g][:].rearrange("p (s x) -> p s x", x=P)
            on_pe = (cc in (5, 11))
            if on_pe:
                cs = csp.tile([128, 1024], F32, tag="cs_ps", name="cs_ps")
            else:
                cs = ep.tile([128, 1024], F32, tag="cs", name="cs")
            for k in range(2):
                c = 2 * cc + k
                Cv = c_tiles[(g, c)][:].rearrange("p (s x) -> p s x", x=SEG)[
                    :, :, 1 : 1 + P
                ]
                if on_pe:
                    nc.tensor.matmul(cs[:, k * 512 : (k + 1) * 512], lhsT=ID[:],
                                     rhs=Cv, start=True, stop=False,
                                     skip_group_check=True)
                    nc.tensor.matmul(cs[:, k * 512 : (k + 1) * 512], lhsT=ones1[:],
                                     rhs=negm512[g][:], start=False, stop=True,
                                     skip_group_check=True)
                else:
                    ov = cs[:, k * 512 : (k + 1) * 512].rearrange(
                        "p (s x) -> p s x", x=P)
                    nc.vector.tensor_tensor(ov, Cv, mv, op=ALU.subtract)
            e = ep.tile([128, 1024], F32, tag="e", name="e")
            nc.scalar.activation(e[:], cs[:], AF.Exp, bias=0.0, scale=-1.0)
            for k in range(2):
                c = 2 * cc + k
                for j in range(2 * NSEG):
                    nc.tensor.matmul(
                        ws[j][:],
                        lhsT=e[:, k * 512 + j * 128 : k * 512 + (j + 1) * 128],
                        rhs=TP_sb[:, c, :],
                        start=(c == 0),
                        stop=(c == NCH - 1),
                        skip_group_check=True,
                    )

        def p4_out(g):
            # all lanes in parallel: out[q, j] = (x_nat . ws[j][:, 0:64]) / ws[j][:, 64]
            ws = ws_tiles[g]
            nrg = sm.tile([128, 2 * NSEG], F32, tag="nrg", name="nrg")
            dcol = sm.tile([128, 2 * NSEG], F32, tag="dcol", name="dcol")
            for j in range(2 * NSEG):
                prod = sm.tile([128, D], F32, tag="prod", name="prod")
                nc.vector.tensor_tensor(
                    prod[:], XN[:, g * 2 * NSEG + j, :], ws[j][:, 0:D], op=ALU.mult
                )
                nc.vector.tensor_reduce(
                    nrg[:, j : j + 1], prod[:], axis=AX.X, op=ALU.add
                )
                nc.scalar.copy(dcol[:, j : j + 1], ws[j][:, D : D + 1])
            rden = sm.tile([128, 2 * NSEG], F32, tag="rden", name="rden")
            nc.vector.reciprocal(rden[:], dcol[:])
            ocol = sm.tile([128, 2 * NSEG], F32, tag="ocol", name="ocol")
            nc.vector.tensor_tensor(ocol[:], nrg[:], rden[:], op=ALU.mult)
            nc.sync.dma_start(
                out_dram[g * NSEG : (g + 1) * NSEG, :].rearrange(
                    "b (h q) -> q b h", q=128
                ),
                ocol[:].rearrange("q (b h) -> q b h", h=2),
            )

        for g in range(NG):
            rm_state[g] = (
                [rmp.tile([128, NSEG * SEG], F32, tag=f"rmA{g}_{i}", name=f"rmA{g}_{i}")
                 for i in (0, 1)],
                0,
                None,
            )

        # software-pipelined emission:
        #  - a few g1 phase-1 chunks bridge g0's min-finalize
        #  - held-back g0 phase-2 pairs bridge g1's min-finalize
        import os as _os
        HOLD = int(_os.environ.get('KHOLD', 2))
        EARLY = int(_os.environ.get('KEARLY', 4))
        for c in range(NCH):
            p1_chunk(0, c)
        for c in range(EARLY):
            p1_chunk(1, c)
        finalize(0)
        nxt = EARLY
        for cc in range(NCH // 2 - HOLD):
            if nxt < NCH:
                p1_chunk(1, nxt)
                nxt += 1
            if nxt < NCH:
                p1_chunk(1, nxt)
                nxt += 1
            p2_pair(0, cc)
        while nxt < NCH:
            p1_chunk(1, nxt)
            nxt += 1
        finalize(1)
        for cc in range(NCH // 2 - HOLD, NCH // 2):
            p2_pair(0, cc)
        p4_out(0)
        for cc in range(NCH // 2):
            p2_pair(1, cc)
        p4_out(1)


_CACHE = {}


def _get_nc():
    if "nc" not in _CACHE:
        nc = bacc.Bacc(
            "TRN2",
            target_bir_lowering=False,
            debug=False,
            enable_asserts=False,
            num_devices=NCORES,
        )
        with tile.TileContext(nc) as tc:
            build_program(tc)
        nc.compile()
        _CACHE["nc"] = nc
    return _CACHE["nc"]


def _make_in_maps(data, targets, task_pool):
    data = np.ascontiguousarray(data, dtype=np.float32)
    targets = np.ascontiguousarray(targets, dtype=np.float32)
    task_pool = np.ascontiguousarray(task_pool, dtype=np.float32)
    W = task_pool[:, :, 0]  # (T, D)
    w_aug = np.concatenate(
        [W.T, -np.ones((1, T), np.float32)], axis=0
    )  # (65, T)


    tp_aug = np.concatenate([W, np.ones((T, 1), np.float32)], axis=1)  # (T, 65)
    ident = np.eye(128, dtype=np.float32)
    in_maps = []
    for core in range(NCORES):
        xa = np.empty((DA, BLOC * P), np.float32)
        for j in range(BLOC):
            b = core * BLOC + j
            xa[0:D, j * P : (j + 1) * P] = data[b].T
            xa[D, j * P : (j + 1) * P] = targets[b]
        xn = np.ascontiguousarray(
            data[core * BLOC : (core + 1) * BLOC].reshape(BLOC * P, D)
        )
        in_maps.append(
            {"w_aug": w_aug, "tp_aug": tp_aug, "x_aug": xa, "ident": ident,
             "x_nat": xn}
        )
    return in_maps


def run(data, targets, task_pool, trace=False):
    nc = _get_nc()
    in_maps = _make_in_maps(data, targets, task_pool)
    res = bass_utils.run_bass_kernel_spmd(
        nc, in_maps, core_ids=list(range(NCORES)), trace=trace
    )
    out = np.empty((B, P), np.float32)
    for core in range(NCORES):
        out[core * BLOC : (core + 1) * BLOC] = res.results[core]["out"]
    return out, res


def kernel(data, targets, task_pool):
    out, _ = run(data, targets, task_pool)
    return out

